# revision 4
# baseline (speedup 1.0000x reference)
"""ConvSTFT kernel for Trainium2 (Bass/Tile), data-parallel over batch on 8 cores.

Structure (v2, tuned against the TimelineSim instruction-cost model):
- 3-level DIF split of the 1024-pt windowed DFT: 22 matmuls/batch of 501
  columns each (16 odd-bin, 4 bins-4t+2, 1 bins-8u, 1 bins-8u+4), all rhs
  bf16 with fp32 PSUM accumulation.
- Host lays the signal out transposed (xst[b, p, j] = xp[b, p + 64 j]) so
  the stride-5 column views feed the window stage without gathers.
- Stage 1 (window+butterfly): 8 tensor-scalar ops split across DVE/Act/Pool
  per a tuned per-batch assignment; stages 1-3 butterflies as wide DVE
  tensor-tensor ops over chunk-packed tiles (2x 16-bit DVE mode).
- Output is packed per batch into one [128, 4016] staging tile
  (blocks g0 | g1 | E | sd, planes at +502) and shipped with two large
  contiguous DMAs per batch (the final batch drains piecewise to shorten
  the tail); the host unscrambles the packed blocks, interleaves re/im,
  and computes the bin-512 row exactly in fp64.
- Batch 0 is split into three frame-jobs fed by quartered xs0 DMAs to
  shorten the prologue; tiny warmup matmuls start the PE busy-streak so
  real matmuls dispatch at full clock (p-state ramp).
- All DMAs are issued from the sync (SP) queue in a tuned global order.

Known-good constraints: Pool (gpsimd) must not read PSUM (neuronxcc
rejects it); DMA access patterns are limited to 3 dims; DMA contiguous
runs must be >= 512B for full bandwidth.
"""

import numpy as np
from contextlib import ExitStack

import concourse.bass as bass
import concourse.tile as tile
from concourse import bacc, mybir

# problem constants (hardcoded per harness contract)
B, T = 32, 160000
NCORES = 8
BPC = B // NCORES
HOP, NFFT = 320, 1024
BINS, F = 513, 501
PAD = NFFT // 2
JC = 2560                    # xst columns (covers j = 5*500 + 14, mult of 16)
L = 127 + 64 * (JC - 1) + 1  # padded xp length backing xst
BF16 = mybir.dt.bfloat16
S2 = 512 * F                 # plane stride in out_dev elements
FTW = 2832                   # ft cols: 16 win + 1024 g0 + 1024 g1 + 512 sd + 256 E
NWARM = 15                   # tiny warmup matmuls (keep PE streak alive)
B0_JOBS = [(0, 160), (160, 165), (325, 176)]
DMA_ORDER = ["q1", "g0", "win", "q2", "q34", "x1", "g1", "x2", "sdE", "x3"]
XSPL = [864, 1712]

_STATE: dict = {}

# per-job window-TS engine assignment: lists of chunk ids 0..7 (0-3 lo, 4-7 hi)
_TS_STEADY = {"dve": [0, 1], "act": [2], "pool": [3, 4, 5, 6, 7]}
_TS_LATEB = {"dve": [0, 1], "act": [], "pool": [2, 3, 4, 5, 6, 7]}
_TS_B0 = {"dve": [0, 1, 2, 3], "act": [6, 7], "pool": [4, 5]}
_TS_B0J = {"dve": [], "act": [0, 1, 6, 7], "pool": [2, 3, 4, 5]}
_TS_B1 = {"dve": [0, 1], "act": [2, 3], "pool": [4, 5, 6, 7]}


def _build_nc():
    nc = bacc.Bacc(
        "TRN2", target_bir_lowering=False, debug=False, num_devices=NCORES
    )
    f32 = mybir.dt.float32
    add, sub = mybir.AluOpType.add, mybir.AluOpType.subtract
    xst = nc.dram_tensor("xst", [BPC, 128, JC], BF16, kind="ExternalInput").ap()
    ft = nc.dram_tensor("ft", [128, FTW], BF16, kind="ExternalInput").ap()
    out = nc.dram_tensor("out", [BPC, 128, 4016], BF16, kind="ExternalOutput").ap()

    with tile.TileContext(nc) as tc, ExitStack() as ctx:
        const_pool = ctx.enter_context(tc.tile_pool(name="const", bufs=1))
        xs_pool = ctx.enter_context(tc.tile_pool(name="xs", bufs=1))
        bf_pool = ctx.enter_context(tc.tile_pool(name="bf", bufs=2))
        st_pool = ctx.enter_context(tc.tile_pool(name="st", bufs=2))
        acc_pool = ctx.enter_context(tc.tile_pool(name="acc", bufs=1, space="PSUM"))
        wu_pool = ctx.enter_context(tc.tile_pool(name="wu", bufs=1))

        # PE p-state warmup: tiny matmuls keep the busy-streak alive through
        # the startup DMA window so real matmuls dispatch at full clock
        dummy = wu_pool.tile([128, 64], BF16, tag="dummy")
        dummy2 = wu_pool.tile([128, 1], BF16, tag="dummy2")
        nc.gpsimd.memset(dummy[:], 0)
        # pull the one-time LoadActFuncSet off the critical path
        nc.scalar.copy(dummy2[:], dummy[:, 0:1])
        pw = acc_pool.tile([128, 1024], f32, tag="ps3")
        for _ in range(NWARM):
            nc.tensor.matmul(
                pw[0:64, 0:64], dummy[:], dummy[:], start=True, stop=True
            )

        ft_sb = const_pool.tile([128, FTW], BF16, tag="ft")
        xs_t = []
        for b in range(BPC):
            xs_b = xs_pool.tile([128, JC], BF16, tag=f"xs{b}", name=f"xs{b}")
            xs_t.append(xs_b)

        # startup DMAs, one queue = strict order: window first, then xs0 in
        # quarters interleaved with the lhs blocks so batch-0 quarter-jobs
        # start as early as possible. Outputs are issued later, same queue.
        dma_parts = {
            "win": (ft_sb[:, 0:16], ft[:, 0:16]),
            "q1": (xs_t[0][:, 0:XSPL[0]], xst[0, :, 0:XSPL[0]]),
            "g0": (ft_sb[:, 16:1040], ft[:, 16:1040]),
            "q2": (xs_t[0][:, XSPL[0]:XSPL[1]], xst[0, :, XSPL[0]:XSPL[1]]),
            "q34": (xs_t[0][:, XSPL[1]:JC], xst[0, :, XSPL[1]:JC]),
            "g1": (ft_sb[:, 1040:2064], ft[:, 1040:2064]),
            "x1": (xs_t[1][:], xst[1]),
            "sdE": (ft_sb[:, 2064:FTW], ft[:, 2064:FTW]),
            "x2": (xs_t[2][:], xst[2]),
            "x3": (xs_t[3][:], xst[3]),
        }
        for part in DMA_ORDER:
            nc.sync.dma_start(*dma_parts[part])

        f32w = lambda o: ft_sb[:, o:o + 2].bitcast(f32)
        wv = [f32w(2 * c) for c in range(8)]        # wl0-3, wh0-4 scalars

        def emit_ts(b, xs, u, t, f0, nf, asn):
            for eng_name, chunks in asn.items():
                eng = {"dve": nc.vector, "act": nc.scalar,
                       "pool": nc.gpsimd}[eng_name]
                for c in chunks:
                    dst = (u if c < 4 else t)[:, F * (c % 4) + f0:
                                              F * (c % 4) + f0 + nf]
                    src = xs[:, 2 * c + 5 * f0: 2 * c + 5 * (f0 + nf): 5]
                    if eng_name == "act":
                        eng.mul(dst, src, wv[c])
                    else:
                        eng.tensor_scalar_mul(dst, src, wv[c])

        def emit_tt(b, bt, f0, nf, stage1_only=False, stage23_only=False):
            u, t, d, s, ss, sd, sss, ssd = bt
            u4 = u[:].rearrange("p (c f) -> p c f", c=4)[:, :, f0:f0 + nf]
            t4 = t[:].rearrange("p (c f) -> p c f", c=4)[:, :, f0:f0 + nf]
            d4 = d[:].rearrange("p (c f) -> p c f", c=4)[:, :, f0:f0 + nf]
            s4 = s[:].rearrange("p (c f) -> p c f", c=4)[:, :, f0:f0 + nf]
            if not stage23_only:
                nc.vector.tensor_tensor(d4, u4, t4, sub)
                nc.vector.tensor_tensor(s4, u4, t4, add)
            if stage1_only:
                return
            slo = s[:].rearrange("p (c f) -> p c f", c=4)[:, 0:2, f0:f0 + nf]
            shi = s[:].rearrange("p (c f) -> p c f", c=4)[:, 2:4, f0:f0 + nf]
            ss2 = ss[:].rearrange("p (c f) -> p c f", c=2)[:, :, f0:f0 + nf]
            sd2 = sd[:].rearrange("p (c f) -> p c f", c=2)[:, :, f0:f0 + nf]
            nc.vector.tensor_tensor(ss2, slo, shi, add)
            nc.vector.tensor_tensor(sd2, slo, shi, sub)
            sslo = ss[:, f0:f0 + nf]
            sshi = ss[:, F + f0:F + f0 + nf]
            nc.vector.tensor_tensor(sss[:, f0:f0 + nf], sslo, sshi, add)
            nc.vector.tensor_tensor(ssd[:, f0:f0 + nf], sslo, sshi, sub)

        def emit_mm(b, bt, ps_g, f0, nf, part=None):
            u, t, d, s, ss, sd, sss, ssd = bt
            for g in ([] if part == "even" else range(2)):  # odd bins
                ps = ps_g[g]
                for c in range(4):
                    lr = ft_sb[:, 16 + 1024 * g + 256 * c:
                               16 + 1024 * g + 256 * c + 128]
                    li = ft_sb[:, 16 + 1024 * g + 256 * c + 128:
                               16 + 1024 * g + 256 * c + 256]
                    rhs = d[:, F * c + f0: F * c + f0 + nf]
                    nc.tensor.matmul(ps[:, f0:f0 + nf], lr, rhs,
                                     start=(c == 0), stop=(c == 3))
                    nc.tensor.matmul(ps[:, 512 + f0:512 + f0 + nf], li, rhs,
                                     start=(c == 0), stop=(c == 3))
            if part == "odd":
                return
            ps = ps_g[3]            # E group: sss | ssd
            nc.tensor.matmul(ps[:, f0:f0 + nf], ft_sb[:, 2576:2704],
                             sss[:, f0:f0 + nf], start=True, stop=True)
            nc.tensor.matmul(ps[:, 512 + f0:512 + f0 + nf],
                             ft_sb[:, 2704:2832],
                             ssd[:, f0:f0 + nf], start=True, stop=True)
            ps = ps_g[2]            # sd group
            for c in range(2):
                lr = ft_sb[:, 2064 + 256 * c: 2064 + 256 * c + 128]
                li = ft_sb[:, 2064 + 256 * c + 128: 2064 + 256 * c + 256]
                rhs = sd[:, F * c + f0: F * c + f0 + nf]
                nc.tensor.matmul(ps[:, f0:f0 + nf], lr, rhs,
                                 start=(c == 0), stop=(c == 1))
                nc.tensor.matmul(ps[:, 512 + f0:512 + f0 + nf], li, rhs,
                                 start=(c == 0), stop=(c == 1))

        def emit_drains(b, ps_g):
            lastb = b == BPC - 1
            sta = st_pool.tile([128, 4016], BF16, tag="sta")
            sa = sta[:].rearrange("p (g i f) -> p g i f", g=4, i=2)[:, :, :, 0:F]
            # packed col blocks: g0 | g1 | E | sd
            def ps3(g):
                return ps_g[g][:].rearrange("p (i f) -> p i f", i=2)[:, :, 0:F]
            nc.scalar.copy(sa[:, 0], ps3(0))
            if lastb:
                nc.sync.dma_start(out[b, :, 0:1004], sta[:, 0:1004])
                nc.vector.tensor_copy(sa[:, 1], ps3(1))
                nc.sync.dma_start(out[b, :, 1004:2008], sta[:, 1004:2008])
            else:
                nc.scalar.copy(sa[:, 1], ps3(1))
                nc.sync.dma_start(out[b, :, 0:2008], sta[:, 0:2008])
            if not lastb:
                nc.scalar.copy(sa[:, 2], ps3(3))
                nc.scalar.copy(sa[:, 3], ps3(2))
                nc.sync.dma_start(out[b, :, 2008:4016], sta[:, 2008:4016])
                return
            nc.scalar.copy(sa[:, 2], ps3(3))
            nc.sync.dma_start(out[b, :, 2008:3012], sta[:, 2008:3012])
            sd3 = sa[:, 3]
            nc.scalar.copy(sd3[:, 0:1], ps3(2)[:, 0:1])
            nc.sync.dma_start(out[b, :, 3012:3514], sta[:, 3012:3514])
            nc.vector.tensor_copy(sd3[:, 1:2], ps3(2)[:, 1:2])
            nc.sync.dma_start(out[b, :, 3514:4016], sta[:, 3514:4016])

        prev = None
        for b in range(BPC):
            xs = xs_t[b]
            u = bf_pool.tile([128, 4 * F], BF16, tag="u")
            t = bf_pool.tile([128, 4 * F], BF16, tag="t")
            jobs = B0_JOBS if b == 0 else [(0, F)]
            # emission order: TS(b) -> drains(b-1) -> TT(b) -> mm(b), so each
            # engine queue sees the next batch's inputs before old drains
            if b > 0:
                emit_ts(b, xs, u, t, 0, F, _TS_B1 if b == 1 else _TS_STEADY)
            if prev is not None:
                emit_drains(*prev)
            d = bf_pool.tile([128, 4 * F], BF16, tag="d")
            s = bf_pool.tile([128, 4 * F], BF16, tag="s")
            ss = bf_pool.tile([128, 2 * F], BF16, tag="ss")
            sd = bf_pool.tile([128, 2 * F], BF16, tag="sd")
            sss = bf_pool.tile([128, F], BF16, tag="sss")
            ssd = bf_pool.tile([128, F], BF16, tag="ssd")
            bt = (u, t, d, s, ss, sd, sss, ssd)
            ps_g = []
            for g in range(4):
                ps_t = acc_pool.tile([128, 1024], f32, tag=f"ps{g}", name=f"ps{g}")
                ps_g.append(ps_t)
            for ji, (f0, nf) in enumerate(jobs):
                if b == 0:
                    asn = _TS_B0 if ji == 0 else _TS_B0J
                    emit_ts(b, xs, u, t, f0, nf, asn)
                emit_tt(b, bt, f0, nf)
                emit_mm(b, bt, ps_g, f0, nf)
            prev = (b, ps_g)
        emit_drains(*prev)

    nc.compile()
    return nc


def _host_prep_basis():
    """Build ft [128, FTW]: window scalars + all DIF lhs blocks."""
    n = np.arange(800)
    window = 0.5 * (1.0 - np.cos(2.0 * np.pi * n / 799.0))
    w = np.zeros(NFFT)
    w[112:912] = window

    ftc = np.zeros((128, FTW))
    p = np.arange(128)[:, None]
    j = np.arange(128)[None, :]

    def dft_block(qbase, kof, kstep):
        ang = 2.0 * np.pi * ((qbase + p) * (kof + kstep * j)) / NFFT
        return np.cos(ang), -np.sin(ang)

    for g in range(2):          # odd bins 2m+1
        for c in range(4):
            cosb, sinb = dft_block(128 * c, 2 * (128 * g) + 1, 2)
            base = 16 + 1024 * g + 256 * c
            ftc[:, base:base + 128] = cosb
            ftc[:, base + 128:base + 256] = sinb
    for c in range(2):          # bins 4t+2
        cosb, sinb = dft_block(128 * c, 2, 4)
        ftc[:, 2064 + 256 * c:2064 + 256 * c + 128] = cosb
        ftc[:, 2064 + 256 * c + 128:2064 + 256 * c + 256] = sinb
    j64 = np.arange(64)[None, :]
    ang = 2.0 * np.pi * (p * (8 * j64)) / NFFT      # bins 8u
    ftc[:, 2576:2640] = np.cos(ang)
    ftc[:, 2640:2704] = -np.sin(ang)
    ang = 2.0 * np.pi * (p * (4 + 8 * j64)) / NFFT  # bins 8u+4
    ftc[:, 2704:2768] = np.cos(ang)
    ftc[:, 2768:2832] = -np.sin(ang)
    return ftc, w


def _get_exec():
    if "exec" in _STATE:
        return _STATE["exec"]

    from concourse import bass2jax

    nc = _build_nc()

    def run(in_maps):
        res = bass2jax.run_bass_via_pjrt(nc, in_maps, n_cores=NCORES)
        return np.concatenate([np.asarray(r["out"]) for r in res], axis=0)

    _STATE["exec"] = run
    return run


def _prep_inputs(x: np.ndarray):
    import ml_dtypes

    x = np.asarray(x, np.float32)
    xp_all = np.zeros((B, L), ml_dtypes.bfloat16)
    xp_all[:, PAD:PAD + T] = x.astype(ml_dtypes.bfloat16)
    sb = xp_all.strides
    xst = np.ascontiguousarray(np.lib.stride_tricks.as_strided(
        xp_all, (B, 128, JC), (sb[0], sb[1], 64 * sb[1])))

    if "ft" not in _STATE:
        ftc, w = _host_prep_basis()
        ftb = ftc.astype(ml_dtypes.bfloat16)
        w8 = np.stack(
            [w[128 * c:128 * c + 128] for c in range(8)], axis=1
        ).astype(np.float32)                       # [128, 8]
        ftb.view(np.uint16)[:, 0:16] = w8.view(np.uint16)
        _STATE["ft"] = np.ascontiguousarray(ftb)
        _STATE["g512"] = ((-1.0) ** np.arange(NFFT)) * w
    ft = _STATE["ft"]

    in_maps = [
        {"xst": xst[BPC * c:BPC * (c + 1)], "ft": ft}
        for c in range(NCORES)
    ]
    return in_maps


def _host_bin512(x: np.ndarray):
    """Exact fp64 bin-512 real row: re512[b, f] = sum_n (-1)^n w_n xp[320f+n]."""
    x = np.asarray(x, np.float64)
    xp = np.zeros((B, T + 2 * PAD))
    xp[:, PAD:PAD + T] = x
    sb = xp.strides
    frames = np.lib.stride_tricks.as_strided(
        xp, (B, F, NFFT), (sb[0], HOP * sb[1], sb[1]))
    return (frames @ _STATE["g512"]).astype(np.float32)   # [B, F]


def kernel(x: np.ndarray, basis: np.ndarray) -> np.ndarray:
    run = _get_exec()
    in_maps = _prep_inputs(x)
    dev = run(in_maps).astype(np.float32)          # [B, 128, 4016] packed
    full = np.empty((B, BINS, F, 2), np.float32)
    # packed col blocks (1004 cols each): g0 | g1 | E | sd; planes at +502
    for i in range(2):
        full[:, 1:257:2, :, i] = dev[:, :, 502 * i: 502 * i + 501]
        full[:, 257:513:2, :, i] = dev[:, :, 1004 + 502 * i: 1004 + 502 * i + 501]
        full[:, 2:513:4, :, i] = dev[:, :, 3012 + 502 * i: 3012 + 502 * i + 501]
        half = dev[:, 64 * i:64 * i + 64, :]
        full[:, 0:512:8, :, i] = half[:, :, 2008:2509]
        full[:, 4:512:8, :, i] = half[:, :, 2510:3011]
    full[:, 512, :, 0] = _host_bin512(x)
    full[:, 512, :, 1] = 0.0
    return full


# revision 5
# speedup vs baseline: 1.0167x; 1.0167x over previous
"""ConvSTFT kernel for Trainium2 (Bass/Tile), data-parallel over batch on 8 cores.

Structure (v2, tuned against the TimelineSim instruction-cost model):
- 3-level DIF split of the 1024-pt windowed DFT: 22 matmuls/batch of 501
  columns each (16 odd-bin, 4 bins-4t+2, 1 bins-8u, 1 bins-8u+4), all rhs
  bf16 with fp32 PSUM accumulation.
- Host lays the signal out transposed (xst[b, p, j] = xp[b, p + 64 j]) so
  the stride-5 column views feed the window stage without gathers.
- Stage 1 (window+butterfly): 8 tensor-scalar ops split across DVE/Act/Pool
  per a tuned per-batch assignment; stages 1-3 butterflies as wide DVE
  tensor-tensor ops over chunk-packed tiles (2x 16-bit DVE mode).
- Output is packed per batch into one [128, 4016] staging tile
  (blocks g0 | g1 | E | sd, planes at +502) and shipped with two large
  contiguous DMAs per batch (the final batch drains piecewise to shorten
  the tail); the host unscrambles the packed blocks, interleaves re/im,
  and computes the bin-512 row exactly in fp64.
- Batch 0 is split into three frame-jobs fed by quartered xs0 DMAs to
  shorten the prologue; tiny warmup matmuls start the PE busy-streak so
  real matmuls dispatch at full clock (p-state ramp).
- All DMAs are issued from the sync (SP) queue in a tuned global order.

Known-good constraints: Pool (gpsimd) must not read PSUM (neuronxcc
rejects it); DMA access patterns are limited to 3 dims; DMA contiguous
runs must be >= 512B for full bandwidth.
"""

import numpy as np
from contextlib import ExitStack

import concourse.bass as bass
import concourse.tile as tile
from concourse import bacc, mybir

# problem constants (hardcoded per harness contract)
B, T = 32, 160000
NCORES = 8
BPC = B // NCORES
HOP, NFFT = 320, 1024
BINS, F = 513, 501
PAD = NFFT // 2
JC = 2560                    # xst columns (covers j = 5*500 + 14, mult of 16)
L = 127 + 64 * (JC - 1) + 1  # padded xp length backing xst
BF16 = mybir.dt.bfloat16
S2 = 512 * F                 # plane stride in out_dev elements
FTW = 2816                   # ft cols: 1024 g0 + 1024 g1 + 512 sd + 256 E
NWARM = 15                   # tiny warmup matmuls (keep PE streak alive)
B0_JOBS = [(0, 160), (160, 165), (325, 176)]
DMA_ORDER = ["q1", "g0", "q2", "q34", "x1", "g1", "x2", "sdE", "x3"]
XSPL = [864, 1712]

_STATE: dict = {}

# per-job window-TS engine assignment: lists of chunk ids 0..7 (0-3 lo, 4-7 hi)
_TS_STEADY = {"dve": [0, 1], "act": [2], "pool": [3, 4, 5, 6, 7]}
_TS_LATEB = {"dve": [0, 1], "act": [], "pool": [2, 3, 4, 5, 6, 7]}
_TS_B0 = {"dve": [0, 1, 2, 3], "act": [6, 7], "pool": [4, 5]}
_TS_B0J = {"dve": [], "act": [0, 1, 6, 7], "pool": [2, 3, 4, 5]}
_TS_B1 = {"dve": [0, 1], "act": [2, 3], "pool": [4, 5, 6, 7]}


def _build_nc():
    nc = bacc.Bacc(
        "TRN2", target_bir_lowering=False, debug=False, num_devices=NCORES
    )
    f32 = mybir.dt.float32
    add, sub = mybir.AluOpType.add, mybir.AluOpType.subtract
    xst = nc.dram_tensor("xst", [BPC, 128, JC], BF16, kind="ExternalInput").ap()
    ft = nc.dram_tensor("ft", [128, FTW], BF16, kind="ExternalInput").ap()
    out = nc.dram_tensor("out", [BPC, 128, 4016], BF16, kind="ExternalOutput").ap()

    with tile.TileContext(nc) as tc, ExitStack() as ctx:
        const_pool = ctx.enter_context(tc.tile_pool(name="const", bufs=1))
        xs_pool = ctx.enter_context(tc.tile_pool(name="xs", bufs=1))
        bf_pool = ctx.enter_context(tc.tile_pool(name="bf", bufs=2))
        st_pool = ctx.enter_context(tc.tile_pool(name="st", bufs=2))
        acc_pool = ctx.enter_context(tc.tile_pool(name="acc", bufs=1, space="PSUM"))
        wu_pool = ctx.enter_context(tc.tile_pool(name="wu", bufs=1))

        # PE p-state warmup: tiny matmuls keep the busy-streak alive through
        # the startup DMA window so real matmuls dispatch at full clock
        dummy = wu_pool.tile([128, 64], BF16, tag="dummy")
        dummy2 = wu_pool.tile([128, 1], BF16, tag="dummy2")
        nc.gpsimd.memset(dummy[:], 0)
        # pull the one-time LoadActFuncSet off the critical path
        nc.scalar.copy(dummy2[:], dummy[:, 0:1])
        pw = acc_pool.tile([128, 1024], f32, tag="ps3")
        for _ in range(NWARM):
            nc.tensor.matmul(
                pw[0:64, 0:64], dummy[:], dummy[:], start=True, stop=True
            )

        ft_sb = const_pool.tile([128, FTW], BF16, tag="ft")
        xs_t = []
        for b in range(BPC):
            xs_b = xs_pool.tile([128, JC], BF16, tag=f"xs{b}", name=f"xs{b}")
            xs_t.append(xs_b)

        # startup DMAs, one queue = strict order: window first, then xs0 in
        # quarters interleaved with the lhs blocks so batch-0 quarter-jobs
        # start as early as possible. Outputs are issued later, same queue.
        dma_parts = {
            "q1": (xs_t[0][:, 0:XSPL[0]], xst[0, :, 0:XSPL[0]]),
            "g0": (ft_sb[:, 0:1024], ft[:, 0:1024]),
            "q2": (xs_t[0][:, XSPL[0]:XSPL[1]], xst[0, :, XSPL[0]:XSPL[1]]),
            "q34": (xs_t[0][:, XSPL[1]:JC], xst[0, :, XSPL[1]:JC]),
            "g1": (ft_sb[:, 1024:2048], ft[:, 1024:2048]),
            "x1": (xs_t[1][:], xst[1]),
            "sdE": (ft_sb[:, 2048:FTW], ft[:, 2048:FTW]),
            "x2": (xs_t[2][:], xst[2]),
            "x3": (xs_t[3][:], xst[3]),
        }
        for part in DMA_ORDER:
            nc.sync.dma_start(*dma_parts[part])

        f32w = lambda o: xs_t[0][:, o:o + 2].bitcast(f32)
        wv = [f32w(2 * c) for c in range(8)]        # wl0-3, wh0-4 scalars

        def emit_ts(b, xs, u, t, f0, nf, asn):
            for eng_name, chunks in asn.items():
                eng = {"dve": nc.vector, "act": nc.scalar,
                       "pool": nc.gpsimd}[eng_name]
                for c in chunks:
                    dst = (u if c < 4 else t)[:, F * (c % 4) + f0:
                                              F * (c % 4) + f0 + nf]
                    src = xs[:, 16 + 2 * c + 5 * f0: 16 + 2 * c + 5 * (f0 + nf): 5]
                    if eng_name == "act":
                        eng.mul(dst, src, wv[c])
                    else:
                        eng.tensor_scalar_mul(dst, src, wv[c])

        def emit_tt(b, bt, f0, nf, stage1_only=False, stage23_only=False):
            u, t, d, s, ss, sd, sss, ssd = bt
            u4 = u[:].rearrange("p (c f) -> p c f", c=4)[:, :, f0:f0 + nf]
            t4 = t[:].rearrange("p (c f) -> p c f", c=4)[:, :, f0:f0 + nf]
            d4 = d[:].rearrange("p (c f) -> p c f", c=4)[:, :, f0:f0 + nf]
            s4 = s[:].rearrange("p (c f) -> p c f", c=4)[:, :, f0:f0 + nf]
            if not stage23_only:
                nc.vector.tensor_tensor(d4, u4, t4, sub)
                nc.vector.tensor_tensor(s4, u4, t4, add)
            if stage1_only:
                return
            slo = s[:].rearrange("p (c f) -> p c f", c=4)[:, 0:2, f0:f0 + nf]
            shi = s[:].rearrange("p (c f) -> p c f", c=4)[:, 2:4, f0:f0 + nf]
            ss2 = ss[:].rearrange("p (c f) -> p c f", c=2)[:, :, f0:f0 + nf]
            sd2 = sd[:].rearrange("p (c f) -> p c f", c=2)[:, :, f0:f0 + nf]
            nc.vector.tensor_tensor(ss2, slo, shi, add)
            nc.vector.tensor_tensor(sd2, slo, shi, sub)
            sslo = ss[:, f0:f0 + nf]
            sshi = ss[:, F + f0:F + f0 + nf]
            nc.vector.tensor_tensor(sss[:, f0:f0 + nf], sslo, sshi, add)
            nc.vector.tensor_tensor(ssd[:, f0:f0 + nf], sslo, sshi, sub)

        def emit_mm(b, bt, ps_g, f0, nf, part=None):
            u, t, d, s, ss, sd, sss, ssd = bt
            for g in ([] if part == "even" else range(2)):  # odd bins
                ps = ps_g[g]
                for c in range(4):
                    lr = ft_sb[:, 1024 * g + 256 * c:
                               1024 * g + 256 * c + 128]
                    li = ft_sb[:, 1024 * g + 256 * c + 128:
                               1024 * g + 256 * c + 256]
                    rhs = d[:, F * c + f0: F * c + f0 + nf]
                    nc.tensor.matmul(ps[:, f0:f0 + nf], lr, rhs,
                                     start=(c == 0), stop=(c == 3))
                    nc.tensor.matmul(ps[:, 512 + f0:512 + f0 + nf], li, rhs,
                                     start=(c == 0), stop=(c == 3))
            if part == "odd":
                return
            ps = ps_g[3]            # E group: sss | ssd
            nc.tensor.matmul(ps[:, f0:f0 + nf], ft_sb[:, 2560:2688],
                             sss[:, f0:f0 + nf], start=True, stop=True)
            nc.tensor.matmul(ps[:, 512 + f0:512 + f0 + nf],
                             ft_sb[:, 2688:2816],
                             ssd[:, f0:f0 + nf], start=True, stop=True)
            ps = ps_g[2]            # sd group
            for c in range(2):
                lr = ft_sb[:, 2048 + 256 * c: 2048 + 256 * c + 128]
                li = ft_sb[:, 2048 + 256 * c + 128: 2048 + 256 * c + 256]
                rhs = sd[:, F * c + f0: F * c + f0 + nf]
                nc.tensor.matmul(ps[:, f0:f0 + nf], lr, rhs,
                                 start=(c == 0), stop=(c == 1))
                nc.tensor.matmul(ps[:, 512 + f0:512 + f0 + nf], li, rhs,
                                 start=(c == 0), stop=(c == 1))

        def emit_drains(b, ps_g):
            lastb = b == BPC - 1
            sta = st_pool.tile([128, 4016], BF16, tag="sta")
            sa = sta[:].rearrange("p (g i f) -> p g i f", g=4, i=2)[:, :, :, 0:F]
            # packed col blocks: g0 | g1 | E | sd
            def ps3(g):
                return ps_g[g][:].rearrange("p (i f) -> p i f", i=2)[:, :, 0:F]
            nc.scalar.copy(sa[:, 0], ps3(0))
            if lastb:
                nc.sync.dma_start(out[b, :, 0:1004], sta[:, 0:1004])
                nc.vector.tensor_copy(sa[:, 1], ps3(1))
                nc.sync.dma_start(out[b, :, 1004:2008], sta[:, 1004:2008])
            else:
                nc.scalar.copy(sa[:, 1], ps3(1))
                nc.sync.dma_start(out[b, :, 0:2008], sta[:, 0:2008])
            if not lastb:
                nc.scalar.copy(sa[:, 2], ps3(3))
                nc.scalar.copy(sa[:, 3], ps3(2))
                nc.sync.dma_start(out[b, :, 2008:4016], sta[:, 2008:4016])
                return
            nc.scalar.copy(sa[:, 2], ps3(3))
            nc.sync.dma_start(out[b, :, 2008:3012], sta[:, 2008:3012])
            sd3 = sa[:, 3]
            nc.scalar.copy(sd3[:, 0:1], ps3(2)[:, 0:1])
            nc.sync.dma_start(out[b, :, 3012:3514], sta[:, 3012:3514])
            nc.vector.tensor_copy(sd3[:, 1:2], ps3(2)[:, 1:2])
            nc.sync.dma_start(out[b, :, 3514:4016], sta[:, 3514:4016])

        prev = None
        for b in range(BPC):
            xs = xs_t[b]
            u = bf_pool.tile([128, 4 * F], BF16, tag="u")
            t = bf_pool.tile([128, 4 * F], BF16, tag="t")
            jobs = B0_JOBS if b == 0 else [(0, F)]
            # emission order: TS(b) -> drains(b-1) -> TT(b) -> mm(b), so each
            # engine queue sees the next batch's inputs before old drains
            if b > 0:
                emit_ts(b, xs, u, t, 0, F, _TS_B1 if b == 1 else _TS_STEADY)
            if prev is not None:
                emit_drains(*prev)
            d = bf_pool.tile([128, 4 * F], BF16, tag="d")
            s = bf_pool.tile([128, 4 * F], BF16, tag="s")
            ss = bf_pool.tile([128, 2 * F], BF16, tag="ss")
            sd = bf_pool.tile([128, 2 * F], BF16, tag="sd")
            sss = bf_pool.tile([128, F], BF16, tag="sss")
            ssd = bf_pool.tile([128, F], BF16, tag="ssd")
            bt = (u, t, d, s, ss, sd, sss, ssd)
            ps_g = []
            for g in range(4):
                ps_t = acc_pool.tile([128, 1024], f32, tag=f"ps{g}", name=f"ps{g}")
                ps_g.append(ps_t)
            for ji, (f0, nf) in enumerate(jobs):
                if b == 0:
                    asn = _TS_B0 if ji == 0 else _TS_B0J
                    emit_ts(b, xs, u, t, f0, nf, asn)
                emit_tt(b, bt, f0, nf)
                emit_mm(b, bt, ps_g, f0, nf)
            prev = (b, ps_g)
        emit_drains(*prev)

    nc.compile()
    return nc


def _host_prep_basis():
    """Build ft [128, FTW]: all DIF lhs blocks (window rides in xst)."""
    n = np.arange(800)
    window = 0.5 * (1.0 - np.cos(2.0 * np.pi * n / 799.0))
    w = np.zeros(NFFT)
    w[112:912] = window

    ftc = np.zeros((128, FTW))
    p = np.arange(128)[:, None]
    j = np.arange(128)[None, :]

    def dft_block(qbase, kof, kstep):
        ang = 2.0 * np.pi * ((qbase + p) * (kof + kstep * j)) / NFFT
        return np.cos(ang), -np.sin(ang)

    for g in range(2):          # odd bins 2m+1
        for c in range(4):
            cosb, sinb = dft_block(128 * c, 2 * (128 * g) + 1, 2)
            base = 1024 * g + 256 * c
            ftc[:, base:base + 128] = cosb
            ftc[:, base + 128:base + 256] = sinb
    for c in range(2):          # bins 4t+2
        cosb, sinb = dft_block(128 * c, 2, 4)
        ftc[:, 2048 + 256 * c:2048 + 256 * c + 128] = cosb
        ftc[:, 2048 + 256 * c + 128:2048 + 256 * c + 256] = sinb
    j64 = np.arange(64)[None, :]
    ang = 2.0 * np.pi * (p * (8 * j64)) / NFFT      # bins 8u
    ftc[:, 2560:2624] = np.cos(ang)
    ftc[:, 2624:2688] = -np.sin(ang)
    ang = 2.0 * np.pi * (p * (4 + 8 * j64)) / NFFT  # bins 8u+4
    ftc[:, 2688:2752] = np.cos(ang)
    ftc[:, 2752:2816] = -np.sin(ang)
    return ftc, w


def _get_exec():
    if "exec" in _STATE:
        return _STATE["exec"]

    from concourse import bass2jax

    nc = _build_nc()

    def run(in_maps):
        res = bass2jax.run_bass_via_pjrt(nc, in_maps, n_cores=NCORES)
        return np.concatenate([np.asarray(r["out"]) for r in res], axis=0)

    _STATE["exec"] = run
    return run


def _prep_inputs(x: np.ndarray):
    import ml_dtypes

    x = np.asarray(x, np.float32)
    xp_all = np.zeros((B, L), ml_dtypes.bfloat16)
    xp_all[:, PAD:PAD + T] = x.astype(ml_dtypes.bfloat16)
    sb = xp_all.strides
    xsig = np.lib.stride_tricks.as_strided(
        xp_all, (B, 128, JC - 16), (sb[0], sb[1], 64 * sb[1]))

    if "ft" not in _STATE:
        ftc, w = _host_prep_basis()
        _STATE["ft"] = np.ascontiguousarray(ftc.astype(ml_dtypes.bfloat16))
        # window scalars as raw fp32 bits in 16 bf16 cols (xst prefix)
        w8 = np.stack(
            [w[128 * c:128 * c + 128] for c in range(8)], axis=1
        ).astype(np.float32)                       # [128, 8]
        wpre = np.zeros((128, 16), ml_dtypes.bfloat16)
        wpre.view(np.uint16)[:] = w8.view(np.uint16)
        _STATE["wpre"] = wpre
        _STATE["g512"] = ((-1.0) ** np.arange(NFFT)) * w
    ft = _STATE["ft"]

    xst = np.empty((B, 128, JC), ml_dtypes.bfloat16)
    xst[:, :, 0:16] = _STATE["wpre"]
    xst[:, :, 16:] = xsig
    in_maps = [
        {"xst": xst[BPC * c:BPC * (c + 1)], "ft": ft}
        for c in range(NCORES)
    ]
    return in_maps


def _host_bin512(x: np.ndarray):
    """Exact fp64 bin-512 real row: re512[b, f] = sum_n (-1)^n w_n xp[320f+n]."""
    x = np.asarray(x, np.float64)
    xp = np.zeros((B, T + 2 * PAD))
    xp[:, PAD:PAD + T] = x
    sb = xp.strides
    frames = np.lib.stride_tricks.as_strided(
        xp, (B, F, NFFT), (sb[0], HOP * sb[1], sb[1]))
    return (frames @ _STATE["g512"]).astype(np.float32)   # [B, F]


def kernel(x: np.ndarray, basis: np.ndarray) -> np.ndarray:
    run = _get_exec()
    in_maps = _prep_inputs(x)
    dev = run(in_maps).astype(np.float32)          # [B, 128, 4016] packed
    full = np.empty((B, BINS, F, 2), np.float32)
    # packed col blocks (1004 cols each): g0 | g1 | E | sd; planes at +502
    for i in range(2):
        full[:, 1:257:2, :, i] = dev[:, :, 502 * i: 502 * i + 501]
        full[:, 257:513:2, :, i] = dev[:, :, 1004 + 502 * i: 1004 + 502 * i + 501]
        full[:, 2:513:4, :, i] = dev[:, :, 3012 + 502 * i: 3012 + 502 * i + 501]
        half = dev[:, 64 * i:64 * i + 64, :]
        full[:, 0:512:8, :, i] = half[:, :, 2008:2509]
        full[:, 4:512:8, :, i] = half[:, :, 2510:3011]
    full[:, 512, :, 0] = _host_bin512(x)
    full[:, 512, :, 1] = 0.0
    return full
